# revision 31
# baseline (speedup 1.0000x reference)
"""GIN message-passing (graph-masked autoencoder step) on 8 Trainium2 NeuronCores.

Strategy (node-sharded, feature-major):
  - 50000 nodes split 8 ways (6250/core, padded to 6272 = 49x128-row windows).
    Full feature table replicated per core in DRAM (fp16 for gathers); each
    core owns its node-slice.
  - segment_sum: edges bucketed by dst core/window on host, gathered in bulk
    via dma_gather (int16 idx -> table split in two views), reduced on the
    TensorEngine as X_tile.T @ onehot accumulating into PSUM (transposed
    aggregate, feature-major). Onehot tiles are built on-device with a DVE
    is_equal against an iota constant from 1-float-per-edge slot vectors.
    The GIN self-term is one identity-onehot tile per window fed by a
    contiguous DMA from the core's own slice.
  - GEMMs with pre-transposed weights keep activations [feature x rows], so
    BatchNorm stats are bn_stats/bn_aggr along the free axis, globalized with
    a 2KB AllReduce; normalize+ReLU is one fused ScalarE activation.
  - Per layer: gather+segsum+GEMM1+stats -> AllReduce -> norm+GEMM2+stats ->
    AllReduce -> norm+transpose+write slice -> AllGather (layers 1,2).
  - The tiny 273-node target encoder and the final cosine loss run on host.
"""
import os
import numpy as np
from contextlib import ExitStack

import concourse.bass as bass
import concourse.bacc as bacc
import concourse.tile as tile
import concourse.mybir as mybir
from concourse.bass_utils import run_bass_kernel_spmd
from concourse import library_config

M = 8          # cores
D = 256        # feature dim
W = 128        # window rows
L = 3          # layers
F32 = mybir.dt.float32
I16 = mybir.dt.int16

# gather dtype (validated: fp16 gathers give ~5e-6 final rel err)
DT = mybir.dt.float16
DT_NP = np.float16
GATHER_GROUP = int(os.environ.get("KERNEL_GG", "1"))  # windows per dma_gather call
GROUP_TILE_BUDGET = 64  # max gathered tiles per (group, half) - bounds SBUF
GPOOL_BUFS = int(os.environ.get("KERNEL_GBUFS", "6"))

LAST_EXEC_NS = None
LAST_PROFILE = None


# --------------------------------------------------------------------------
# host-side graph structure
# --------------------------------------------------------------------------
class Structure:
    pass


def build_structure(src, dst, n_nodes, npc, split):
    assert n_nodes == M * npc
    rpc = ((npc + W - 1) // W) * W
    wpc = rpc // W
    s = Structure()
    s.n_nodes, s.npc, s.rpc, s.wpc, s.split = n_nodes, npc, rpc, wpc, split
    s.npad = M * rpc

    src = np.asarray(src, np.int64)
    dst = np.asarray(dst, np.int64)
    c = dst // npc
    ld = dst % npc
    w = ld // W
    slot = ld % W
    srcrow = rpc * (src // npc) + src % npc
    half = (srcrow >= split).astype(np.int64)
    idxval = srcrow - split * half
    assert split <= 32768 and (s.npad - split) <= 32768
    assert idxval.max(initial=0) < 32768

    key = (c * wpc + w) * 2 + half
    counts = np.bincount(key, minlength=M * wpc * 2).reshape(M, wpc, 2)
    maxcnt = counts.max(axis=0)
    T = -(-maxcnt // W)
    s.T_lo = T[:, 0].copy()
    s.T_hi = T[:, 1].copy()
    s.tiles_w = s.T_lo + s.T_hi + 1
    s.tile_off = np.concatenate([[0], np.cumsum(s.tiles_w)]).astype(np.int64)
    s.tiles_tot = int(s.tile_off[-1])
    s.lo_off = np.concatenate([[0], np.cumsum(s.T_lo * W)]).astype(np.int64)
    s.hi_off = np.concatenate([[0], np.cumsum(s.T_hi * W)]).astype(np.int64)
    s.n_lo = int(s.lo_off[-1])
    s.n_hi = int(s.hi_off[-1])

    # sort by (bucket, src row) so gather addresses ascend within each call
    # (ascending HBM reads are ~15% faster); dvec slots follow ranks, so any
    # within-bucket order is valid.
    order = np.lexsort((idxval, key))
    ranks = np.empty_like(order)
    sec_start = np.concatenate([[0], np.cumsum(counts.reshape(-1))])
    ranks[order] = np.arange(len(order)) - np.repeat(sec_start[:-1], counts.reshape(-1))

    s.idx_lo = np.zeros((M, max(s.n_lo, 16)), np.int16)
    s.idx_hi = np.zeros((M, max(s.n_hi, 16)), np.int16)
    s.dvec = np.full((M, W, s.tiles_tot), 255.0, np.float32)
    selfcol = s.tile_off[:-1] + s.T_lo + s.T_hi
    s.dvec[:, :, selfcol] = np.arange(W, dtype=np.float32)[None, :, None]

    for name, hsel, idxarr, off, tbase in (
        ("lo", half == 0, s.idx_lo, s.lo_off, s.tile_off[:-1]),
        ("hi", half == 1, s.idx_hi, s.hi_off, s.tile_off[:-1] + s.T_lo),
    ):
        e = np.flatnonzero(hsel)
        idxarr[c[e], off[w[e]] + ranks[e]] = idxval[e].astype(np.int16)
        s.dvec[c[e], ranks[e] % W, tbase[w[e]] + ranks[e] // W] = slot[e]
    return s


def idx_sbuf_layout(flat):
    n = flat.shape[-1]
    assert n % 16 == 0
    a = flat.reshape(n // 16, 16).T
    return np.ascontiguousarray(np.tile(a, (8, 1)))


def pad_table(h, npc, rpc):
    n, d = h.shape
    out = np.zeros((M, rpc, d), h.dtype)
    out[:, :npc] = h.reshape(M, npc, d)
    return out.reshape(M * rpc, d)


# --------------------------------------------------------------------------
# bass program
# --------------------------------------------------------------------------
def build_program(s):
    npc, rpc, wpc, split, npad = s.npc, s.rpc, s.wpc, s.split, s.npad
    n_lo_c = max(s.n_lo, 16) // 16
    n_hi_c = max(s.n_hi, 16) // 16
    maxT = int(s.tiles_w.max())

    # window groups for gather calls: target GATHER_GROUP windows, capped by
    # a per-half tile budget so skewed degree distributions still fit SBUF
    groups = []
    g = 0
    while g < wpc:
        e = g + 1
        while (e < min(g + GATHER_GROUP, wpc)
               and (s.lo_off[e + 1] - s.lo_off[g]) // W <= GROUP_TILE_BUDGET
               and (s.hi_off[e + 1] - s.hi_off[g]) // W <= GROUP_TILE_BUDGET):
            e += 1
        groups.append(list(range(g, e)))
        g = e
    glo = [int(s.lo_off[g[-1] + 1] - s.lo_off[g[0]]) for g in groups]
    ghi = [int(s.hi_off[g[-1] + 1] - s.hi_off[g[0]]) for g in groups]
    max_glo = max(glo) // W if s.n_lo else 0
    max_ghi = max(ghi) // W if s.n_hi else 0

    ONECORE = bool(int(os.environ.get("KERNEL_1CORE", "0")))
    ABL = set(x for x in os.environ.get("KERNEL_ABLATE", "").split(",") if x)
    PH = os.environ.get("KERNEL_PHASES", "ABC")
    # CUT=N keeps only the first N pipeline stages of each layer (timing
    # attribution; every kept stage has all producers kept, so no
    # read-of-unallocated-tile). L0SRC=1 makes every layer gather from the
    # layer-0 input (needed for CUT<11 which skips the AllGather).
    CUT = int(os.environ.get("KERNEL_CUT", "99"))
    L0SRC = bool(int(os.environ.get("KERNEL_L0SRC", "0")))
    NQ = int(os.environ.get("KERNEL_NQUEUES", "4"))
    nc = bacc.Bacc("TRN2", target_bir_lowering=False, debug=False,
                   num_devices=1 if ONECORE else M, num_swdge_queues=NQ)

    h0_full = nc.dram_tensor("h0_full", [npad, D], DT, kind="ExternalInput")
    h0_slice = nc.dram_tensor("h0_slice", [rpc, D], DT, kind="ExternalInput")
    idx_lo_d = nc.dram_tensor("idx_lo", [128, n_lo_c], I16, kind="ExternalInput")
    idx_hi_d = nc.dram_tensor("idx_hi", [128, n_hi_c], I16, kind="ExternalInput")
    dvec_d = nc.dram_tensor("dvec", [W, s.tiles_tot], DT, kind="ExternalInput")
    iota_d = nc.dram_tensor("iota", [128, 128], DT, kind="ExternalInput")
    ident_d = nc.dram_tensor("ident", [128, 128], F32, kind="ExternalInput")
    identdt_d = nc.dram_tensor("identdt", [128, 128], DT, kind="ExternalInput")
    w1t_d = nc.dram_tensor("w1t", [L, 2, 2, 128, 128], F32, kind="ExternalInput")
    w2t_d = nc.dram_tensor("w2t", [L, 2, 2, 128, 128], F32, kind="ExternalInput")
    gb_d = nc.dram_tensor("gb", [L, 2, 2, 2, 128], F32, kind="ExternalInput")
    h3_d = nc.dram_tensor("h3", [rpc, D], F32, kind="ExternalOutput")
    debug = bool(int(os.environ.get("KERNEL_DEBUG_TAPS", "0")))
    if debug:
        dbg_agg = nc.dram_tensor("dbg_agg", [128, 2, rpc], F32, kind="ExternalOutput")
        dbg_t = nc.dram_tensor("dbg_t", [2, 128, rpc], F32, kind="ExternalOutput")
        dbg_m = nc.dram_tensor("dbg_m", [2, 128, rpc], F32, kind="ExternalOutput")
        dbg_kc = nc.dram_tensor("dbg_kc", [2, 128, 4], F32, kind="ExternalOutput")

    rg = [list(range(M))]
    if ONECORE:
        rg = [[0]]

    def wcnt(w):  # real rows in window
        return max(0, min(W, npc - w * W))

    _gq = [0]

    def next_q():  # round-robin gathers over the SWDGE queues
        q = _gq[0] % NQ
        _gq[0] += 1
        return q

    with tile.TileContext(nc) as tc, ExitStack() as ctx:
        nc.gpsimd.load_library(library_config.mlp)
        singles = ctx.enter_context(tc.tile_pool(name="singles", bufs=1))
        gpool = ctx.enter_context(tc.tile_pool(name="gather", bufs=GPOOL_BUFS))
        spool = ctx.enter_context(tc.tile_pool(name="selfp", bufs=3))
        opool = ctx.enter_context(tc.tile_pool(
            name="oh", bufs=int(os.environ.get("KERNEL_OBUFS", "3"))))
        evac = ctx.enter_context(tc.tile_pool(name="evac", bufs=3))
        hout = ctx.enter_context(tc.tile_pool(name="hout", bufs=3))
        stp = ctx.enter_context(tc.tile_pool(name="stats", bufs=3))
        wst = ctx.enter_context(tc.tile_pool(name="winstats", bufs=2))
        pagg_p = ctx.enter_context(tc.tile_pool(name="pagg", bufs=2, space="PSUM"))
        pgem_p = ctx.enter_context(tc.tile_pool(name="pgem", bufs=2, space="PSUM"))
        ptr_p = ctx.enter_context(tc.tile_pool(name="ptr", bufs=2, space="PSUM"))
        dram = ctx.enter_context(tc.tile_pool(name="dram", bufs=2, space="DRAM"))
        dram1 = ctx.enter_context(tc.tile_pool(name="dram1", bufs=2, space="DRAM"))

        # persistent SBUF state
        idxlo_sb = singles.tile([128, n_lo_c], I16)
        idxhi_sb = singles.tile([128, n_hi_c], I16)
        dvec_sb = singles.tile([W, s.tiles_tot], DT)
        iota_sb = singles.tile([128, 128], DT)
        ident_sb = singles.tile([128, 128], F32)
        identdt_sb = singles.tile([128, 128], DT)
        w1t_sb = singles.tile([128, L * 4, 128], F32)
        w2t_sb = singles.tile([128, L * 4, 128], F32)
        gb_sb = singles.tile([128, L * 8], F32)
        eps_sb = singles.tile([128, 1], F32)
        actT = [singles.tile([128, rpc], F32, tag=f"actT{c}", name=f"actT{c}") for c in range(2)]

        nc.sync.dma_start(idxlo_sb[:], idx_lo_d[:])
        nc.sync.dma_start(idxhi_sb[:], idx_hi_d[:])
        nc.sync.dma_start(dvec_sb[:], dvec_d[:])
        nc.sync.dma_start(iota_sb[:], iota_d[:])
        nc.sync.dma_start(ident_sb[:], ident_d[:])
        nc.sync.dma_start(identdt_sb[:], identdt_d[:])
        nc.sync.dma_start(w1t_sb[:], w1t_d.ap().rearrange("l i o p f -> p (l i o) f"))
        nc.sync.dma_start(w2t_sb[:], w2t_d.ap().rearrange("l i o p f -> p (l i o) f"))
        nc.sync.dma_start(gb_sb[:], gb_d.ap().rearrange("l b c g p -> p (l b c g)"))
        nc.vector.memset(eps_sb[:], 1e-5)

        def alloc_layer_bufs(rep):
            hf = [None, None]
            sl = [None, None]
            for l in range(2):
                hf[l] = dram1.tile([npad, D], DT, tag="hfull", name=f"hfull{l}r{rep}",
                                   addr_space="Local" if ONECORE else "Shared")
                sl[l] = dram1.tile([rpc, D], DT, tag="slice", name=f"slice{l}r{rep}")
            return hf, sl

        def bn_apply_coeffs(l, bn, st):
            if "bn" in ABL:
                return stp.tile([128, 4], F32, tag="kc", name="kcabl")
            """AllReduce exact [Sx, Sxx]; return kc tile [128,4] = [k0,k1,c0,c1].

            bn_stats rows are (cnt_e, mean_e, cnt*var_e, cnt_o, mean_o, cnt*var_o)
            per window; combine exactly: Sx = sum cnt*mean, Sxx = sum
            (cnt*var + cnt*mean^2)."""
            pack = stp.tile([128, 4], F32, tag="pack")
            for c in range(2):
                a = wst.tile([128, wpc], F32, tag="bna")
                b = wst.tile([128, wpc], F32, tag="bnb")
                sxx = wst.tile([128, wpc], F32, tag="bnsxx")
                t1 = wst.tile([128, wpc], F32, tag="bnt1")
                nc.vector.tensor_mul(a[:], st[c][:, :, 0], st[c][:, :, 1])
                nc.vector.tensor_mul(b[:], st[c][:, :, 3], st[c][:, :, 4])
                nc.vector.tensor_add(sxx[:], st[c][:, :, 2], st[c][:, :, 5])
                nc.vector.tensor_mul(t1[:], a[:], st[c][:, :, 1])
                nc.vector.tensor_add(sxx[:], sxx[:], t1[:])
                nc.vector.tensor_mul(t1[:], b[:], st[c][:, :, 4])
                nc.vector.tensor_add(sxx[:], sxx[:], t1[:])
                nc.vector.tensor_add(a[:], a[:], b[:])
                nc.vector.reduce_sum(pack[:, 2 * c: 2 * c + 1], a[:],
                                     axis=mybir.AxisListType.X)
                nc.vector.reduce_sum(pack[:, 2 * c + 1: 2 * c + 2], sxx[:],
                                     axis=mybir.AxisListType.X)
            arin = dram.tile([128, 4], F32, tag="arin")
            arout = dram.tile([128, 4], F32, tag="arout", addr_space="Shared")
            nc.sync.dma_start(arin[:], pack[:])
            if ONECORE:
                nc.sync.dma_start(arout[:], arin[:])
            else:
                nc.gpsimd.collective_compute(
                    "AllReduce", mybir.AluOpType.add, replica_groups=rg,
                    ins=[arin.opt()], outs=[arout.opt()])
            ar = stp.tile([128, 4], F32, tag="ar")
            nc.sync.dma_start(ar[:], arout[:])
            kc = stp.tile([128, 4], F32, tag="kc")
            mg = stp.tile([128, 2], F32, tag="mg")
            inv_n = 1.0 / s.n_nodes
            for c in range(2):
                # global mean / E[x^2]
                nc.scalar.mul(mg[:, c: c + 1], ar[:, 2 * c: 2 * c + 1], inv_n)
                nc.scalar.mul(ar[:, 2 * c + 1: 2 * c + 2], ar[:, 2 * c + 1: 2 * c + 2], inv_n)
                v = stp.tile([128, 1], F32, tag="var")
                nc.vector.tensor_mul(v[:], mg[:, c: c + 1], mg[:, c: c + 1])
                nc.vector.tensor_tensor(out=v[:], in0=ar[:, 2 * c + 1: 2 * c + 2],
                                        in1=v[:], op=mybir.AluOpType.subtract)
                # sd = sqrt(var + eps); rinv = 1/sd
                nc.scalar.activation(out=v[:], in_=v[:],
                                     func=mybir.ActivationFunctionType.Sqrt,
                                     bias=eps_sb[:], scale=1.0)
                nc.vector.reciprocal(out=v[:], in_=v[:])
                g_ap = gb_sb[:, (((l * 2 + bn) * 2 + c) * 2 + 0): (((l * 2 + bn) * 2 + c) * 2 + 1)]
                b_ap = gb_sb[:, (((l * 2 + bn) * 2 + c) * 2 + 1): (((l * 2 + bn) * 2 + c) * 2 + 2)]
                nc.vector.tensor_mul(kc[:, c: c + 1], g_ap, v[:])
                nc.vector.tensor_mul(v[:], mg[:, c: c + 1], kc[:, c: c + 1])
                nc.vector.tensor_tensor(out=kc[:, 2 + c: 3 + c], in0=b_ap, in1=v[:],
                                        op=mybir.AluOpType.subtract)
            return kc

        repeat = int(os.environ.get("KERNEL_REPEAT", "1"))
        for _rep in range(repeat):
          hfull_t, slice_t = alloc_layer_bufs(_rep)
          for l in range(L):
              hsrc_full = h0_full.ap() if (l == 0 or L0SRC) else hfull_t[l - 1][:]
              hsrc_slice = h0_slice.ap() if (l == 0 or L0SRC) else slice_t[l - 1][:]
              st1 = [wst.tile([128, wpc, 6], F32, tag=f"st1{c}", name=f"st1_{c}") for c in range(2)]
              st2 = [wst.tile([128, wpc, 6], F32, tag=f"st2{c}", name=f"st2_{c}") for c in range(2)]

              # ---------------- phase A ----------------
              for gi, grp in enumerate(groups):
                  xlo = gpool.tile([128, max_glo, D], DT, tag="xlo", name="xlo") if glo[gi] else None
                  xhi = gpool.tile([128, max_ghi, D], DT, tag="xhi", name="xhi") if ghi[gi] else None
                  if glo[gi] and "gather" not in ABL and CUT >= 1:
                      c0 = int(s.lo_off[grp[0]]) // 16
                      nc.gpsimd.dma_gather(
                          xlo[:, : glo[gi] // W, :], hsrc_full[0:split, :],
                          idxlo_sb[:, c0: c0 + glo[gi] // 16], glo[gi], glo[gi], D,
                          single_packet=False, queue_num=next_q())
                  if ghi[gi] and "gather" not in ABL and CUT >= 1:
                      c0 = int(s.hi_off[grp[0]]) // 16
                      nc.gpsimd.dma_gather(
                          xhi[:, : ghi[gi] // W, :], hsrc_full[split:npad, :],
                          idxhi_sb[:, c0: c0 + ghi[gi] // 16], ghi[gi], ghi[gi], D,
                          single_packet=False, queue_num=next_q())
                  for w in grp:
                      tw = int(s.tiles_w[w])
                      to = int(s.tile_off[w])
                      oh = opool.tile([128, maxT, 128], DT, tag="oh")
                      if "oh" not in ABL and CUT >= 1:
                       nc.vector.tensor_tensor(
                          out=oh[:, :tw - 1, :],
                          in0=dvec_sb[:, to: to + tw - 1].to_broadcast([W, tw - 1, 128]),
                          in1=iota_sb[:].rearrange("p (t f) -> p t f", t=1).broadcast_to([128, tw - 1, 128]),
                          op=mybir.AluOpType.is_equal)
                      xself = spool.tile([128, D], DT, tag="xself")
                      if "self" not in ABL and CUT >= 1:
                       nc.sync.dma_start(xself[:], hsrc_slice[w * W:(w + 1) * W, :])
                      # segment-sum matmuls: aggT[i] = sum_t X_t[:, chunk i].T @ onehot_t
                      pagg = pagg_p.tile([128, 2, 128], F32, tag="pagg")
                      lo0 = (int(s.lo_off[w]) - int(s.lo_off[grp[0]])) // W
                      hi0 = (int(s.hi_off[w]) - int(s.hi_off[grp[0]])) // W
                      srcs = ([(xlo, lo0 + t, t) for t in range(int(s.T_lo[w]))]
                              + [(xhi, hi0 + t, int(s.T_lo[w]) + t) for t in range(int(s.T_hi[w]))]
                              + [(xself, None, tw - 1)])
                      for i in range(2 if "segmm" not in ABL and CUT >= 2 else 0):
                          for k, (buf, tloc, tcol) in enumerate(srcs):
                              lhsT = (buf[:, i * 128:(i + 1) * 128] if tloc is None
                                      else buf[:, tloc, i * 128:(i + 1) * 128])
                              rhs = identdt_sb[:] if tloc is None else oh[:, tcol, :]
                              nc.tensor.matmul(pagg[:, i, :], lhsT=lhsT, rhs=rhs,
                                               start=(k == 0), stop=(k == len(srcs) - 1))
                      aggT = evac.tile([128, 2, 128], F32, tag="aggT")
                      if "evac" not in ABL and CUT >= 3:
                       nc.scalar.copy(aggT[:], pagg[:])
                      if debug and l == 0:
                          nc.sync.dma_start(dbg_agg[:, :, w * W:(w + 1) * W], aggT[:])
                      # GEMM1: tT[o] = sum_i W1T[i,o].T @ aggT[i]
                      pt = pgem_p.tile([128, 2, 128], F32, tag="pgem")
                      for o in range(2 if "gemm" not in ABL and CUT >= 4 else 0):
                          for i in range(2):
                              nc.tensor.matmul(pt[:, o, :], lhsT=w1t_sb[:, l * 4 + i * 2 + o, :],
                                               rhs=aggT[:, i, :], start=(i == 0), stop=(i == 1))
                      for c in range(2):
                          if "evac" not in ABL and CUT >= 4:
                           nc.scalar.copy(actT[c][:, w * W:(w + 1) * W], pt[:, c, :])
                          if "bn" not in ABL and CUT >= 5:
                           nc.vector.bn_stats(out=st1[c][:, w, :],
                                              in_=actT[c][:, w * W: w * W + wcnt(w)])
                      if debug and l == 0:
                          for c in range(2):
                              nc.sync.dma_start(dbg_t[c, :, w * W:(w + 1) * W],
                                                actT[c][:, w * W:(w + 1) * W])

              if "B" not in PH or CUT < 5:
                  continue
              kc1 = bn_apply_coeffs(l, 0, st1)
              if debug and l == 0:
                  nc.sync.dma_start(dbg_kc[0], kc1[:])

              # ---------------- phase B (512-col strips) ----------------
              SW = 512
              nstrip = (rpc + SW - 1) // SW
              for c in range(2 if "act" not in ABL and CUT >= 6 else 0):
                  nc.scalar.activation(
                      out=actT[c][:], in_=actT[c][:],
                      func=mybir.ActivationFunctionType.Relu,
                      bias=kc1[:, 2 + c: 3 + c], scale=kc1[:, c: c + 1])
              for st_i in range(nstrip if CUT >= 7 else 0):
                  c0s = st_i * SW
                  c1s = min(rpc, c0s + SW)
                  pm = pgem_p.tile([128, 2, SW], F32, tag="pgem")
                  for o in range(2 if "gemm" not in ABL else 0):
                      for i in range(2):
                          nc.tensor.matmul(pm[:, o, : c1s - c0s],
                                           lhsT=w2t_sb[:, l * 4 + i * 2 + o, :],
                                           rhs=actT[i][:, c0s:c1s],
                                           start=(i == 0), stop=(i == 1))
                  for c in range(2):
                      if "evac" not in ABL:
                       nc.scalar.copy(actT[c][:, c0s:c1s], pm[:, c, : c1s - c0s])
              if "bn" not in ABL and CUT >= 8:
                  for c in range(2):
                      for w in range(wpc):
                          nc.vector.bn_stats(out=st2[c][:, w, :],
                                             in_=actT[c][:, w * W: w * W + wcnt(w)])
              if debug and l == 0:
                  for c in range(2):
                      nc.sync.dma_start(dbg_m[c], actT[c][:])

              if "C" not in PH or CUT < 8:
                  continue
              kc2 = bn_apply_coeffs(l, 1, st2)
              if debug and l == 0:
                  nc.sync.dma_start(dbg_kc[1], kc2[:])

              # ---------------- phase C ----------------
              for c in range(2 if "act" not in ABL and CUT >= 9 else 0):
                  nc.scalar.activation(
                      out=actT[c][:], in_=actT[c][:],
                      func=mybir.ActivationFunctionType.Relu,
                      bias=kc2[:, 2 + c: 3 + c], scale=kc2[:, c: c + 1])
              for w in range(wpc if CUT >= 10 else 0):
                  ptr = ptr_p.tile([128, 2, 128], F32, tag="ptr")
                  for c in range(2 if "tr" not in ABL else 0):
                      nc.tensor.transpose(ptr[:, c, :], actT[c][:, w * W:(w + 1) * W],
                                          ident_sb[:])
                  hrow = hout.tile([128, 2, 128], F32, tag="hrow")
                  nc.scalar.copy(hrow[:], ptr[:])
                  if l < L - 1:
                      hdt = hout.tile([128, D], DT, tag="hdt")
                      nc.vector.tensor_copy(hdt[:], hrow[:].rearrange("p a b -> p (a b)"))

                      nc.sync.dma_start(slice_t[l][w * W:(w + 1) * W, :], hdt[:])
                  else:
                      nc.sync.dma_start(h3_d[w * W:(w + 1) * W, :],
                                        hrow[:].rearrange("p a b -> p (a b)"))
              if l < L - 1 and CUT >= 11:
                  if ONECORE:
                      # model AllGather cost as writing the full table locally
                      for mc in range(M):
                          nc.sync.dma_start(hfull_t[l][mc * rpc:(mc + 1) * rpc, :],
                                            slice_t[l][:])
                  else:
                      nc.gpsimd.collective_compute(
                          "AllGather", mybir.AluOpType.bypass, replica_groups=rg,
                          ins=[slice_t[l].opt()], outs=[hfull_t[l].opt()])

    nc.compile()
    return nc


# --------------------------------------------------------------------------
# host-side helpers (small encoder, loss)
# --------------------------------------------------------------------------
def _np_bn(x, g, b):
    mu = x.mean(0)
    var = ((x - mu) ** 2).mean(0)
    return (x - mu) * (1.0 / np.sqrt(var + 1e-5)) * g + b


def _np_encoder(h, src, dst, W1, W2, g1, b1, g2, b2):
    h = h.astype(np.float32)
    for l in range(W1.shape[0]):
        acc = np.zeros_like(h)
        np.add.at(acc, dst, h[src])
        agg = h + acc
        mm = np.maximum(_np_bn(agg @ W1[l].T, g1[l], b1[l]), 0)
        mm = mm @ W2[l].T
        h = np.maximum(_np_bn(mm, g2[l], b2[l]), 0)
    return h


_CACHE = {}


def _get_program(s):
    key = (s.n_nodes, s.npc, s.split, tuple(s.T_lo), tuple(s.T_hi),
           os.environ.get("KERNEL_REPEAT", "1"),
           os.environ.get("KERNEL_1CORE", "0"),
           os.environ.get("KERNEL_ABLATE", ""),
           os.environ.get("KERNEL_PHASES", "ABC"),
           os.environ.get("KERNEL_CUT", "99"),
           os.environ.get("KERNEL_L0SRC", "0"),
           os.environ.get("KERNEL_NQUEUES", "4"),
           os.environ.get("KERNEL_GG", "1"),
           os.environ.get("KERNEL_GBUFS", "6"),
           os.environ.get("KERNEL_OBUFS", "3"))
    if key not in _CACHE:
        _CACHE[key] = build_program(s)
    return _CACHE[key]


def run_encoder_device(s, rem, weights):
    """rem [n_nodes, D] f32; weights dict with W1,W2,g1,b1,g2,b2 [L,...].
    Returns h_final [n_nodes, D] f32."""
    global LAST_EXEC_NS, LAST_PROFILE
    npc, rpc = s.npc, s.rpc
    nc = _get_program(s)

    h0p = pad_table(rem.astype(np.float32), npc, rpc).astype(DT_NP)
    W1, W2 = weights["W1"], weights["W2"]
    w1t = np.zeros((L, 2, 2, 128, 128), np.float32)
    w2t = np.zeros((L, 2, 2, 128, 128), np.float32)
    for l in range(L):
        for i in range(2):
            for o in range(2):
                w1t[l, i, o] = W1[l][o * 128:(o + 1) * 128, i * 128:(i + 1) * 128].T
                w2t[l, i, o] = W2[l][o * 128:(o + 1) * 128, i * 128:(i + 1) * 128].T
    gb = np.zeros((L, 2, 2, 2, 128), np.float32)
    for l in range(L):
        for c in range(2):
            gb[l, 0, c, 0] = weights["g1"][l][c * 128:(c + 1) * 128]
            gb[l, 0, c, 1] = weights["b1"][l][c * 128:(c + 1) * 128]
            gb[l, 1, c, 0] = weights["g2"][l][c * 128:(c + 1) * 128]
            gb[l, 1, c, 1] = weights["b2"][l][c * 128:(c + 1) * 128]
    iota = np.broadcast_to(np.arange(128, dtype=DT_NP), (128, 128)).copy()
    ident = np.eye(128, dtype=np.float32)
    identdt = np.eye(128, dtype=DT_NP)

    in_maps = []
    for c in range(M):
        in_maps.append({
            "h0_full": h0p,
            "h0_slice": np.ascontiguousarray(h0p[c * rpc:(c + 1) * rpc]),
            "idx_lo": idx_sbuf_layout(s.idx_lo[c]),
            "idx_hi": idx_sbuf_layout(s.idx_hi[c]),
            "dvec": s.dvec[c].astype(DT_NP),
            "iota": iota,
            "ident": ident, "identdt": identdt,
            "w1t": w1t, "w2t": w2t, "gb": gb,
        })
    trace = bool(int(os.environ.get("KERNEL_TRACE", "0")))
    res = run_bass_kernel_spmd(nc, in_maps, core_ids=list(range(M)), trace=trace)
    LAST_EXEC_NS = res.exec_time_ns
    LAST_PROFILE = res.profile_json
    h = np.concatenate([res.results[c]["h3"][:npc] for c in range(M)], 0)
    return h


def kernel(feat, enc_mask_token, src, dst, ring_nodes, sub_src, sub_dst,
           on_W1, on_W2, on_g1, on_b1, on_g2, on_b2,
           tg_W1, tg_W2, tg_g1, tg_b1, tg_g2, tg_b2):
    feat = np.asarray(feat, np.float32)
    ring = np.asarray(ring_nodes, np.int64)
    rem = feat.copy()
    rem[ring] = np.asarray(enc_mask_token, np.float32)[0]

    n = feat.shape[0]
    s = build_structure(np.asarray(src), np.asarray(dst), n, n // M, 32768)
    h1 = run_encoder_device(s, rem, dict(W1=np.asarray(on_W1), W2=np.asarray(on_W2),
                                         g1=np.asarray(on_g1), b1=np.asarray(on_b1),
                                         g2=np.asarray(on_g2), b2=np.asarray(on_b2)))

    h2 = _np_encoder(feat[ring], np.asarray(sub_src, np.int64),
                     np.asarray(sub_dst, np.int64),
                     np.asarray(tg_W1), np.asarray(tg_W2), np.asarray(tg_g1),
                     np.asarray(tg_b1), np.asarray(tg_g2), np.asarray(tg_b2))

    x = h1[ring]
    xn = x / np.maximum(np.linalg.norm(x, axis=-1, keepdims=True), 1e-12)
    yn = h2 / np.maximum(np.linalg.norm(h2, axis=-1, keepdims=True), 1e-12)
    return np.float32((1.0 - (xn * yn).sum(-1)).mean())



# revision 54
# speedup vs baseline: 1.5357x; 1.5357x over previous
"""GIN message-passing (graph-masked autoencoder step) on 8 Trainium2 NeuronCores.

Strategy (node-sharded, feature-major):
  - 50000 nodes split 8 ways (6250/core, padded to 6272 = 49x128-row windows).
    Full feature table replicated per core in DRAM (fp16 for gathers); each
    core owns its node-slice.
  - segment_sum: edges bucketed by dst core/window on host, gathered in bulk
    via dma_gather (int16 idx -> table split in two views), reduced on the
    TensorEngine as X_tile.T @ onehot accumulating into PSUM (transposed
    aggregate, feature-major). Onehot tiles are built on-device with a DVE
    is_equal against an iota constant from 1-float-per-edge slot vectors.
    The GIN self-term is one identity-onehot tile per window fed by a
    contiguous DMA from the core's own slice.
  - GEMMs with pre-transposed weights keep activations [feature x rows], so
    BatchNorm stats are bn_stats/bn_aggr along the free axis, globalized with
    a 2KB AllReduce; normalize+ReLU is one fused ScalarE activation.
  - Per layer: gather+segsum+GEMM1+stats -> AllReduce -> norm+GEMM2+stats ->
    AllReduce -> norm+transpose+write slice -> AllGather (layers 1,2).
  - The tiny 273-node target encoder and the final cosine loss run on host.
"""
import os
import numpy as np
from contextlib import ExitStack

import concourse.bass as bass
import concourse.bacc as bacc
import concourse.tile as tile
import concourse.mybir as mybir
from concourse.bass_utils import run_bass_kernel_spmd
from concourse import library_config

M = 8          # cores
D = 256        # feature dim
W = 128        # window rows
L = 3          # layers
F32 = mybir.dt.float32
I16 = mybir.dt.int16

# gather dtype (validated: fp16 gathers give ~5e-6 final rel err)
DT = mybir.dt.float16
DT_NP = np.float16
GATHER_GROUP = int(os.environ.get("KERNEL_GG", "1"))  # windows per dma_gather call
GROUP_TILE_BUDGET = 64  # max gathered tiles per (group, half) - bounds SBUF
GPOOL_BUFS = int(os.environ.get("KERNEL_GBUFS", "6"))

LAST_EXEC_NS = None
LAST_PROFILE = None


# --------------------------------------------------------------------------
# host-side graph structure
# --------------------------------------------------------------------------
class Structure:
    pass


def build_structure(src, dst, n_nodes, npc, split_w):
    """split_w: windows per core in the 'lo' table chunk. The node table is
    stored as two core-interleaved chunk tensors (lo: per-core windows
    [0, split_w), hi: the rest) so each chunk's AllGather is a whole-tensor
    single-writer collective that can start as soon as its windows finish."""
    assert n_nodes == M * npc
    rpc = ((npc + W - 1) // W) * W
    wpc = rpc // W
    s = Structure()
    s.n_nodes, s.npc, s.rpc, s.wpc = n_nodes, npc, rpc, wpc
    s.split_w = split_w
    s.rows_lo = split_w * W          # per-core rows in lo chunk
    s.rows_hi = rpc - s.rows_lo
    s.npad = M * rpc

    src = np.asarray(src, np.int64)
    dst = np.asarray(dst, np.int64)
    c = dst // npc
    ld = dst % npc
    w = ld // W
    slot = ld % W
    srcm = src // npc
    srcr = src % npc
    half = (srcr >= s.rows_lo).astype(np.int64)
    idxval = np.where(half == 0, srcm * s.rows_lo + srcr,
                      srcm * s.rows_hi + (srcr - s.rows_lo))
    assert M * s.rows_lo <= 32768 and M * s.rows_hi <= 32768
    assert idxval.max(initial=0) < 32768

    key = (c * wpc + w) * 2 + half
    counts = np.bincount(key, minlength=M * wpc * 2).reshape(M, wpc, 2)
    maxcnt = counts.max(axis=0)
    T = -(-maxcnt // W)
    s.T_lo = T[:, 0].copy()
    s.T_hi = T[:, 1].copy()
    s.tiles_w = s.T_lo + s.T_hi + 1
    s.tile_off = np.concatenate([[0], np.cumsum(s.tiles_w)]).astype(np.int64)
    s.tiles_tot = int(s.tile_off[-1])
    s.lo_off = np.concatenate([[0], np.cumsum(s.T_lo * W)]).astype(np.int64)
    s.hi_off = np.concatenate([[0], np.cumsum(s.T_hi * W)]).astype(np.int64)
    s.n_lo = int(s.lo_off[-1])
    s.n_hi = int(s.hi_off[-1])

    # sort by (bucket, src row) so gather addresses ascend within each call
    # (ascending HBM reads are ~15% faster); dvec slots follow ranks, so any
    # within-bucket order is valid.
    order = np.lexsort((idxval, key))
    ranks = np.empty_like(order)
    sec_start = np.concatenate([[0], np.cumsum(counts.reshape(-1))])
    ranks[order] = np.arange(len(order)) - np.repeat(sec_start[:-1], counts.reshape(-1))

    s.idx_lo = np.zeros((M, max(s.n_lo, 16)), np.int16)
    s.idx_hi = np.zeros((M, max(s.n_hi, 16)), np.int16)
    s.dvec = np.full((M, W, s.tiles_tot), 255.0, np.float32)
    selfcol = s.tile_off[:-1] + s.T_lo + s.T_hi
    s.dvec[:, :, selfcol] = np.arange(W, dtype=np.float32)[None, :, None]

    for name, hsel, idxarr, off, tbase in (
        ("lo", half == 0, s.idx_lo, s.lo_off, s.tile_off[:-1]),
        ("hi", half == 1, s.idx_hi, s.hi_off, s.tile_off[:-1] + s.T_lo),
    ):
        e = np.flatnonzero(hsel)
        idxarr[c[e], off[w[e]] + ranks[e]] = idxval[e].astype(np.int16)
        s.dvec[c[e], ranks[e] % W, tbase[w[e]] + ranks[e] // W] = slot[e]
    return s


def idx_sbuf_layout(flat):
    n = flat.shape[-1]
    assert n % 16 == 0
    a = flat.reshape(n // 16, 16).T
    return np.ascontiguousarray(np.tile(a, (8, 1)))


def pad_table(h, npc, rpc):
    n, d = h.shape
    out = np.zeros((M, rpc, d), h.dtype)
    out[:, :npc] = h.reshape(M, npc, d)
    return out.reshape(M * rpc, d)


# --------------------------------------------------------------------------
# bass program
# --------------------------------------------------------------------------
def build_program(s):
    npc, rpc, wpc, npad = s.npc, s.rpc, s.wpc, s.npad
    split_w, rows_lo, rows_hi = s.split_w, s.rows_lo, s.rows_hi
    n_lo_c = max(s.n_lo, 16) // 16
    n_hi_c = max(s.n_hi, 16) // 16
    maxT = int(s.tiles_w.max())

    # window groups for gather calls: target GATHER_GROUP windows, capped by
    # a per-half tile budget so skewed degree distributions still fit SBUF
    groups = []
    g = 0
    while g < wpc:
        e = g + 1
        while (e < min(g + GATHER_GROUP, wpc)
               and (s.lo_off[e + 1] - s.lo_off[g]) // W <= GROUP_TILE_BUDGET
               and (s.hi_off[e + 1] - s.hi_off[g]) // W <= GROUP_TILE_BUDGET):
            e += 1
        groups.append(list(range(g, e)))
        g = e
    glo = [int(s.lo_off[g[-1] + 1] - s.lo_off[g[0]]) for g in groups]
    ghi = [int(s.hi_off[g[-1] + 1] - s.hi_off[g[0]]) for g in groups]
    max_glo = max(glo) // W if s.n_lo else 0
    max_ghi = max(ghi) // W if s.n_hi else 0

    ONECORE = bool(int(os.environ.get("KERNEL_1CORE", "0")))
    ABL = set(x for x in os.environ.get("KERNEL_ABLATE", "").split(",") if x)
    PH = os.environ.get("KERNEL_PHASES", "ABC")
    # CUT=N keeps only the first N pipeline stages of each layer (timing
    # attribution; every kept stage has all producers kept, so no
    # read-of-unallocated-tile). L0SRC=1 makes every layer gather from the
    # layer-0 input (needed for CUT<11 which skips the AllGather).
    CUT = int(os.environ.get("KERNEL_CUT", "99"))
    L0SRC = bool(int(os.environ.get("KERNEL_L0SRC", "0")))
    NQ = int(os.environ.get("KERNEL_NQUEUES", "4"))
    AGCH = int(os.environ.get("KERNEL_AGCH", "4"))
    nc = bacc.Bacc("TRN2", target_bir_lowering=False, debug=False,
                   num_devices=1 if ONECORE else M, num_swdge_queues=NQ)

    h0_lo = nc.dram_tensor("h0_lo", [M * rows_lo, D], DT, kind="ExternalInput")
    h0_hi = nc.dram_tensor("h0_hi", [M * rows_hi, D], DT, kind="ExternalInput")
    h0_slice = nc.dram_tensor("h0_slice", [rpc, D], DT, kind="ExternalInput")
    idx_lo_d = nc.dram_tensor("idx_lo", [128, n_lo_c], I16, kind="ExternalInput")
    idx_hi_d = nc.dram_tensor("idx_hi", [128, n_hi_c], I16, kind="ExternalInput")
    dvec_d = nc.dram_tensor("dvec", [W, s.tiles_tot], DT, kind="ExternalInput")
    iota_d = nc.dram_tensor("iota", [128, 128], DT, kind="ExternalInput")
    ident_d = nc.dram_tensor("ident", [128, 128], F32, kind="ExternalInput")
    identdt_d = nc.dram_tensor("identdt", [128, 128], DT, kind="ExternalInput")
    w1t_d = nc.dram_tensor("w1t", [L, 2, 2, 128, 128], F32, kind="ExternalInput")
    w2t_d = nc.dram_tensor("w2t", [L, 2, 2, 128, 128], F32, kind="ExternalInput")
    gb_d = nc.dram_tensor("gb", [L, 2, 2, 2, 128], F32, kind="ExternalInput")
    h3_d = nc.dram_tensor("h3", [rpc, D], F32, kind="ExternalOutput")
    debug = bool(int(os.environ.get("KERNEL_DEBUG_TAPS", "0")))
    if debug:
        dbg_agg = nc.dram_tensor("dbg_agg", [128, 2, rpc], F32, kind="ExternalOutput")
        dbg_t = nc.dram_tensor("dbg_t", [2, 128, rpc], F32, kind="ExternalOutput")
        dbg_m = nc.dram_tensor("dbg_m", [2, 128, rpc], F32, kind="ExternalOutput")
        dbg_kc = nc.dram_tensor("dbg_kc", [2, 128, 4], F32, kind="ExternalOutput")

    rg = [list(range(M))]
    if ONECORE:
        rg = [[0]]

    def wcnt(w):  # real rows in window
        return max(0, min(W, npc - w * W))

    _gq = [0]

    def next_q():  # round-robin gathers over the SWDGE queues
        q = _gq[0] % NQ
        _gq[0] += 1
        return q

    # BN stats in 512-col blocks over the valid columns only (pad cols are
    # all at the end); BN_STATS_FMAX=512 makes this the minimum op count
    SB = 512
    NST = -(-npc // SB)

    def stat_cols(b):
        return b * SB, min((b + 1) * SB, npc)

    with tile.TileContext(nc) as tc, ExitStack() as ctx:
        nc.gpsimd.load_library(library_config.mlp)
        singles = ctx.enter_context(tc.tile_pool(name="singles", bufs=1))
        gpool = ctx.enter_context(tc.tile_pool(name="gather", bufs=GPOOL_BUFS))
        spool = ctx.enter_context(tc.tile_pool(name="selfp", bufs=3))
        opool = ctx.enter_context(tc.tile_pool(
            name="oh", bufs=int(os.environ.get("KERNEL_OBUFS", "3"))))
        evac = ctx.enter_context(tc.tile_pool(name="evac", bufs=3))
        hout = ctx.enter_context(tc.tile_pool(name="hout", bufs=3))
        stp = ctx.enter_context(tc.tile_pool(name="stats", bufs=3))
        wst = ctx.enter_context(tc.tile_pool(name="winstats", bufs=2))
        pagg_p = ctx.enter_context(tc.tile_pool(name="pagg", bufs=2, space="PSUM"))
        pgem_p = ctx.enter_context(tc.tile_pool(name="pgem", bufs=2, space="PSUM"))
        ptr_p = ctx.enter_context(tc.tile_pool(name="ptr", bufs=2, space="PSUM"))
        dram = ctx.enter_context(tc.tile_pool(name="dram", bufs=2, space="DRAM"))
        dram1 = ctx.enter_context(tc.tile_pool(name="dram1", bufs=2, space="DRAM"))

        # persistent SBUF state
        idxlo_sb = singles.tile([128, n_lo_c], I16)
        idxhi_sb = singles.tile([128, n_hi_c], I16)
        dvec_sb = singles.tile([W, s.tiles_tot], DT)
        iota_sb = singles.tile([128, 128], DT)
        ident_sb = singles.tile([128, 128], F32)
        identdt_sb = singles.tile([128, 128], DT)
        w1t_sb = singles.tile([128, L * 4, 128], F32)
        w2t_sb = singles.tile([128, L * 4, 128], F32)
        gb_sb = singles.tile([128, L * 8], F32)
        eps_sb = singles.tile([128, 1], F32)
        actT = [singles.tile([128, rpc], F32, tag=f"actT{c}", name=f"actT{c}") for c in range(2)]

        nc.sync.dma_start(idxlo_sb[:], idx_lo_d[:])
        nc.sync.dma_start(idxhi_sb[:], idx_hi_d[:])
        nc.sync.dma_start(dvec_sb[:], dvec_d[:])
        nc.sync.dma_start(iota_sb[:], iota_d[:])
        nc.sync.dma_start(ident_sb[:], ident_d[:])
        nc.sync.dma_start(identdt_sb[:], identdt_d[:])
        nc.sync.dma_start(w1t_sb[:], w1t_d.ap().rearrange("l i o p f -> p (l i o) f"))
        nc.sync.dma_start(w2t_sb[:], w2t_d.ap().rearrange("l i o p f -> p (l i o) f"))
        nc.sync.dma_start(gb_sb[:], gb_d.ap().rearrange("l b c g p -> p (l b c g)"))
        nc.vector.memset(eps_sb[:], 1e-5)

        def alloc_layer_bufs(rep):
            hflo = [None, None]
            hfhi = [None, None]
            sl = [None, None]
            sp = "Local" if ONECORE else "Shared"
            for l in range(2):
                hflo[l] = dram1.tile([M * rows_lo, D], DT, tag="hflo",
                                     name=f"hflo{l}r{rep}", addr_space=sp)
                hfhi[l] = dram1.tile([M * rows_hi, D], DT, tag="hfhi",
                                     name=f"hfhi{l}r{rep}", addr_space=sp)
                sl[l] = dram1.tile([rpc, D], DT, tag="slice", name=f"slice{l}r{rep}")
            return hflo, hfhi, sl

        def bn_apply_coeffs(l, bn, st):
            if "bn" in ABL:
                return stp.tile([128, 4], F32, tag="kc", name="kcabl")
            """AllReduce exact [Sx, Sxx]; return kc tile [128,4] = [k0,k1,c0,c1].

            bn_stats rows are (cnt_e, mean_e, cnt*var_e, cnt_o, mean_o, cnt*var_o)
            per window; combine exactly: Sx = sum cnt*mean, Sxx = sum
            (cnt*var + cnt*mean^2)."""
            pack = stp.tile([128, 4], F32, tag="pack")
            for c in range(2):
                a = wst.tile([128, NST], F32, tag="bna")
                b = wst.tile([128, NST], F32, tag="bnb")
                sxx = wst.tile([128, NST], F32, tag="bnsxx")
                t1 = wst.tile([128, NST], F32, tag="bnt1")
                nc.vector.tensor_mul(a[:], st[c][:, :, 0], st[c][:, :, 1])
                nc.vector.tensor_mul(b[:], st[c][:, :, 3], st[c][:, :, 4])
                nc.vector.tensor_add(sxx[:], st[c][:, :, 2], st[c][:, :, 5])
                nc.vector.tensor_mul(t1[:], a[:], st[c][:, :, 1])
                nc.vector.tensor_add(sxx[:], sxx[:], t1[:])
                nc.vector.tensor_mul(t1[:], b[:], st[c][:, :, 4])
                nc.vector.tensor_add(sxx[:], sxx[:], t1[:])
                nc.vector.tensor_add(a[:], a[:], b[:])
                nc.vector.reduce_sum(pack[:, 2 * c: 2 * c + 1], a[:],
                                     axis=mybir.AxisListType.X)
                nc.vector.reduce_sum(pack[:, 2 * c + 1: 2 * c + 2], sxx[:],
                                     axis=mybir.AxisListType.X)
            arin = dram.tile([128, 4], F32, tag="arin")
            arout = dram.tile([128, 4], F32, tag="arout", addr_space="Shared")
            nc.sync.dma_start(arin[:], pack[:])
            if ONECORE:
                nc.sync.dma_start(arout[:], arin[:])
            else:
                nc.gpsimd.collective_compute(
                    "AllReduce", mybir.AluOpType.add, replica_groups=rg,
                    ins=[arin.opt()], outs=[arout.opt()])
            ar = stp.tile([128, 4], F32, tag="ar")
            nc.sync.dma_start(ar[:], arout[:])
            kc = stp.tile([128, 4], F32, tag="kc")
            mg = stp.tile([128, 2], F32, tag="mg")
            inv_n = 1.0 / s.n_nodes
            for c in range(2):
                # global mean / E[x^2]
                nc.scalar.mul(mg[:, c: c + 1], ar[:, 2 * c: 2 * c + 1], inv_n)
                nc.scalar.mul(ar[:, 2 * c + 1: 2 * c + 2], ar[:, 2 * c + 1: 2 * c + 2], inv_n)
                v = stp.tile([128, 1], F32, tag="var")
                nc.vector.tensor_mul(v[:], mg[:, c: c + 1], mg[:, c: c + 1])
                nc.vector.tensor_tensor(out=v[:], in0=ar[:, 2 * c + 1: 2 * c + 2],
                                        in1=v[:], op=mybir.AluOpType.subtract)
                # sd = sqrt(var + eps); rinv = 1/sd
                nc.scalar.activation(out=v[:], in_=v[:],
                                     func=mybir.ActivationFunctionType.Sqrt,
                                     bias=eps_sb[:], scale=1.0)
                nc.vector.reciprocal(out=v[:], in_=v[:])
                g_ap = gb_sb[:, (((l * 2 + bn) * 2 + c) * 2 + 0): (((l * 2 + bn) * 2 + c) * 2 + 1)]
                b_ap = gb_sb[:, (((l * 2 + bn) * 2 + c) * 2 + 1): (((l * 2 + bn) * 2 + c) * 2 + 2)]
                nc.vector.tensor_mul(kc[:, c: c + 1], g_ap, v[:])
                nc.vector.tensor_mul(v[:], mg[:, c: c + 1], kc[:, c: c + 1])
                nc.vector.tensor_tensor(out=kc[:, 2 + c: 3 + c], in0=b_ap, in1=v[:],
                                        op=mybir.AluOpType.subtract)
            return kc

        repeat = int(os.environ.get("KERNEL_REPEAT", "1"))
        for _rep in range(repeat):
          hflo_t, hfhi_t, slice_t = alloc_layer_bufs(_rep)
          for l in range(L):
              first = l == 0 or L0SRC
              hsrc_lo = h0_lo.ap() if first else hflo_t[l - 1][:]
              hsrc_hi = h0_hi.ap() if first else hfhi_t[l - 1][:]
              hsrc_slice = h0_slice.ap() if first else slice_t[l - 1][:]
              st1 = [wst.tile([128, NST, 6], F32, tag=f"st1{c}", name=f"st1_{c}") for c in range(2)]
              st2 = [wst.tile([128, NST, 6], F32, tag=f"st2{c}", name=f"st2_{c}") for c in range(2)]

              # ---------------- phase A ----------------
              for gi, grp in enumerate(groups):
                  xlo = gpool.tile([128, max_glo, D], DT, tag="xlo", name="xlo") if glo[gi] else None
                  xhi = gpool.tile([128, max_ghi, D], DT, tag="xhi", name="xhi") if ghi[gi] else None
                  if glo[gi] and "gather" not in ABL and CUT >= 1:
                      c0 = int(s.lo_off[grp[0]]) // 16
                      nc.gpsimd.dma_gather(
                          xlo[:, : glo[gi] // W, :], hsrc_lo[0:M * rows_lo, :],
                          idxlo_sb[:, c0: c0 + glo[gi] // 16], glo[gi], glo[gi], D,
                          single_packet=False, queue_num=next_q())
                  if ghi[gi] and "gather" not in ABL and CUT >= 1:
                      c0 = int(s.hi_off[grp[0]]) // 16
                      nc.gpsimd.dma_gather(
                          xhi[:, : ghi[gi] // W, :], hsrc_hi[0:M * rows_hi, :],
                          idxhi_sb[:, c0: c0 + ghi[gi] // 16], ghi[gi], ghi[gi], D,
                          single_packet=False, queue_num=next_q())
                  for w in grp:
                      tw = int(s.tiles_w[w])
                      to = int(s.tile_off[w])
                      oh = opool.tile([128, maxT, 128], DT, tag="oh")
                      if "oh" not in ABL and CUT >= 1:
                       nc.vector.tensor_tensor(
                          out=oh[:, :tw - 1, :],
                          in0=dvec_sb[:, to: to + tw - 1].to_broadcast([W, tw - 1, 128]),
                          in1=iota_sb[:].rearrange("p (t f) -> p t f", t=1).broadcast_to([128, tw - 1, 128]),
                          op=mybir.AluOpType.is_equal)
                      xself = spool.tile([128, D], DT, tag="xself")
                      if "self" not in ABL and CUT >= 1:
                       nc.sync.dma_start(xself[:], hsrc_slice[w * W:(w + 1) * W, :])
                      # segment-sum matmuls: aggT[i] = sum_t X_t[:, chunk i].T @ onehot_t
                      pagg = pagg_p.tile([128, 2, 128], F32, tag="pagg")
                      lo0 = (int(s.lo_off[w]) - int(s.lo_off[grp[0]])) // W
                      hi0 = (int(s.hi_off[w]) - int(s.hi_off[grp[0]])) // W
                      srcs = ([(xlo, lo0 + t, t) for t in range(int(s.T_lo[w]))]
                              + [(xhi, hi0 + t, int(s.T_lo[w]) + t) for t in range(int(s.T_hi[w]))]
                              + [(xself, None, tw - 1)])
                      for i in range(2 if "segmm" not in ABL and CUT >= 2 else 0):
                          for k, (buf, tloc, tcol) in enumerate(srcs):
                              lhsT = (buf[:, i * 128:(i + 1) * 128] if tloc is None
                                      else buf[:, tloc, i * 128:(i + 1) * 128])
                              rhs = identdt_sb[:] if tloc is None else oh[:, tcol, :]
                              nc.tensor.matmul(pagg[:, i, :], lhsT=lhsT, rhs=rhs,
                                               start=(k == 0), stop=(k == len(srcs) - 1))
                      aggT = evac.tile([128, 2, 128], F32, tag="aggT")
                      if "evac" not in ABL and CUT >= 3:
                       nc.scalar.copy(aggT[:], pagg[:])
                      if debug and l == 0:
                          nc.sync.dma_start(dbg_agg[:, :, w * W:(w + 1) * W], aggT[:])
                      # GEMM1: tT[o] = sum_i W1T[i,o].T @ aggT[i]
                      pt = pgem_p.tile([128, 2, 128], F32, tag="pgem")
                      for o in range(2 if "gemm" not in ABL and CUT >= 4 else 0):
                          for i in range(2):
                              nc.tensor.matmul(pt[:, o, :], lhsT=w1t_sb[:, l * 4 + i * 2 + o, :],
                                               rhs=aggT[:, i, :], start=(i == 0), stop=(i == 1))
                      for c in range(2):
                          if "evac" not in ABL and CUT >= 4:
                           nc.scalar.copy(actT[c][:, w * W:(w + 1) * W], pt[:, c, :])
                      if debug and l == 0:
                          for c in range(2):
                              nc.sync.dma_start(dbg_t[c, :, w * W:(w + 1) * W],
                                                actT[c][:, w * W:(w + 1) * W])

              if "bn" not in ABL and CUT >= 5:
                  for c in range(2):
                      for b in range(NST):
                          b0, b1 = stat_cols(b)
                          nc.vector.bn_stats(out=st1[c][:, b, :],
                                             in_=actT[c][:, b0:b1])

              if "B" not in PH or CUT < 5:
                  continue
              kc1 = bn_apply_coeffs(l, 0, st1)
              if debug and l == 0:
                  nc.sync.dma_start(dbg_kc[0], kc1[:])

              # ---------------- phase B (512-col strips) ----------------
              SW = 512
              nstrip = (rpc + SW - 1) // SW
              # strip-pipelined: act1 -> gemm2 -> evac -> bn2 stats per strip,
              # so ScalarE/PE/DVE overlap instead of full-array barriers
              for st_i in range(nstrip if CUT >= 6 else 0):
                  c0s = st_i * SW
                  c1s = min(rpc, c0s + SW)
                  for c in range(2 if "act" not in ABL else 0):
                      nc.scalar.activation(
                          out=actT[c][:, c0s:c1s], in_=actT[c][:, c0s:c1s],
                          func=mybir.ActivationFunctionType.Relu,
                          bias=kc1[:, 2 + c: 3 + c], scale=kc1[:, c: c + 1])
                  if CUT < 7:
                      continue
                  pm = pgem_p.tile([128, 2, SW], F32, tag="pgem")
                  for o in range(2 if "gemm" not in ABL else 0):
                      for i in range(2):
                          nc.tensor.matmul(pm[:, o, : c1s - c0s],
                                           lhsT=w2t_sb[:, l * 4 + i * 2 + o, :],
                                           rhs=actT[i][:, c0s:c1s],
                                           start=(i == 0), stop=(i == 1))
                  for c in range(2):
                      if "evac" not in ABL:
                       nc.scalar.copy(actT[c][:, c0s:c1s], pm[:, c, : c1s - c0s])
                      if "bn" not in ABL and CUT >= 8 and st_i < NST:
                          b0, b1 = stat_cols(st_i)
                          nc.vector.bn_stats(out=st2[c][:, st_i, :],
                                             in_=actT[c][:, b0:b1])
              if debug and l == 0:
                  for c in range(2):
                      nc.sync.dma_start(dbg_m[c], actT[c][:])

              if "C" not in PH or CUT < 8:
                  continue
              kc2 = bn_apply_coeffs(l, 1, st2)
              if debug and l == 0:
                  nc.sync.dma_start(dbg_kc[1], kc2[:])

              # ---------------- phase C (chunked; AllGather overlaps) -------
              # chunk boundary == table chunk boundary, so each chunk's
              # AllGather is a whole-tensor single-writer collective
              wb = [0, split_w, wpc] if l < L - 1 else [0, wpc]
              for k in range(len(wb) - 1):
                  wa, wz = wb[k], wb[k + 1]
                  if wa == wz:
                      continue
                  agdst = (hflo_t if k == 0 else hfhi_t)[l] if l < L - 1 else None
                  rows_k = rows_lo if k == 0 else rows_hi
                  for c in range(2 if "act" not in ABL and CUT >= 9 else 0):
                      nc.scalar.activation(
                          out=actT[c][:, wa * W: wz * W],
                          in_=actT[c][:, wa * W: wz * W],
                          func=mybir.ActivationFunctionType.Relu,
                          bias=kc2[:, 2 + c: 3 + c], scale=kc2[:, c: c + 1])
                  # 4-window blocks: transpose each window, evacuate PSUM with
                  # one engine op per window, then one DMA per block
                  BL = 4
                  for w0 in range(*((wa, wz, BL) if CUT >= 10 else (0, 0, BL))):
                      w1 = min(w0 + BL, wz)
                      nb = w1 - w0
                      if l < L - 1:
                          hdt = hout.tile([128, BL, D], DT, tag="hdt")
                      else:
                          hdt = hout.tile([128, BL, D], F32, tag="hrow")
                      for w in range(w0, w1):
                          ptr = ptr_p.tile([128, 2, 128], F32, tag="ptr")
                          for c in range(2 if "tr" not in ABL else 0):
                              nc.tensor.transpose(ptr[:, c, :],
                                                  actT[c][:, w * W:(w + 1) * W],
                                                  ident_sb[:])
                          nc.vector.tensor_copy(hdt[:, w - w0, :],
                                                ptr[:].rearrange("p a b -> p (a b)"))
                      dst_d = slice_t[l] if l < L - 1 else h3_d
                      nc.sync.dma_start(
                          dst_d[w0 * W: w1 * W, :].rearrange(
                              "(b p) d -> p b d", p=W),
                          hdt[:, :nb, :])
                  if l < L - 1 and CUT >= 11:
                      if ONECORE:
                          # model AllGather cost as writing the table locally
                          for mc in range(M):
                              nc.sync.dma_start(
                                  agdst[mc * rows_k:(mc + 1) * rows_k, :],
                                  slice_t[l][wa * W: wz * W, :])
                      else:
                          nc.gpsimd.collective_compute(
                              "AllGather", mybir.AluOpType.bypass, replica_groups=rg,
                              ins=[slice_t[l][wa * W: wz * W, :].opt()],
                              outs=[agdst[:].opt()])

    nc.compile()
    return nc


# --------------------------------------------------------------------------
# host-side helpers (small encoder, loss)
# --------------------------------------------------------------------------
def _np_bn(x, g, b):
    mu = x.mean(0)
    var = ((x - mu) ** 2).mean(0)
    return (x - mu) * (1.0 / np.sqrt(var + 1e-5)) * g + b


def _np_encoder(h, src, dst, W1, W2, g1, b1, g2, b2):
    h = h.astype(np.float32)
    for l in range(W1.shape[0]):
        acc = np.zeros_like(h)
        np.add.at(acc, dst, h[src])
        agg = h + acc
        mm = np.maximum(_np_bn(agg @ W1[l].T, g1[l], b1[l]), 0)
        mm = mm @ W2[l].T
        h = np.maximum(_np_bn(mm, g2[l], b2[l]), 0)
    return h


_CACHE = {}


def _get_program(s):
    key = (s.n_nodes, s.npc, s.split_w, tuple(s.T_lo), tuple(s.T_hi),
           os.environ.get("KERNEL_REPEAT", "1"),
           os.environ.get("KERNEL_1CORE", "0"),
           os.environ.get("KERNEL_ABLATE", ""),
           os.environ.get("KERNEL_PHASES", "ABC"),
           os.environ.get("KERNEL_CUT", "99"),
           os.environ.get("KERNEL_L0SRC", "0"),
           os.environ.get("KERNEL_NQUEUES", "4"),
           os.environ.get("KERNEL_GG", "1"),
           os.environ.get("KERNEL_GBUFS", "6"),
           os.environ.get("KERNEL_OBUFS", "3"),
           os.environ.get("KERNEL_AGCH", "4"))
    if key not in _CACHE:
        _CACHE[key] = build_program(s)
    return _CACHE[key]


def run_encoder_device(s, rem, weights):
    """rem [n_nodes, D] f32; weights dict with W1,W2,g1,b1,g2,b2 [L,...].
    Returns h_final [n_nodes, D] f32."""
    global LAST_EXEC_NS, LAST_PROFILE
    npc, rpc = s.npc, s.rpc
    nc = _get_program(s)

    h0p = pad_table(rem.astype(np.float32), npc, rpc).astype(DT_NP)
    W1, W2 = weights["W1"], weights["W2"]
    w1t = np.zeros((L, 2, 2, 128, 128), np.float32)
    w2t = np.zeros((L, 2, 2, 128, 128), np.float32)
    for l in range(L):
        for i in range(2):
            for o in range(2):
                w1t[l, i, o] = W1[l][o * 128:(o + 1) * 128, i * 128:(i + 1) * 128].T
                w2t[l, i, o] = W2[l][o * 128:(o + 1) * 128, i * 128:(i + 1) * 128].T
    gb = np.zeros((L, 2, 2, 2, 128), np.float32)
    for l in range(L):
        for c in range(2):
            gb[l, 0, c, 0] = weights["g1"][l][c * 128:(c + 1) * 128]
            gb[l, 0, c, 1] = weights["b1"][l][c * 128:(c + 1) * 128]
            gb[l, 1, c, 0] = weights["g2"][l][c * 128:(c + 1) * 128]
            gb[l, 1, c, 1] = weights["b2"][l][c * 128:(c + 1) * 128]
    iota = np.broadcast_to(np.arange(128, dtype=DT_NP), (128, 128)).copy()
    ident = np.eye(128, dtype=np.float32)
    identdt = np.eye(128, dtype=DT_NP)

    h3d = h0p.reshape(M, rpc, D)
    h0_lo = np.ascontiguousarray(h3d[:, :s.rows_lo]).reshape(M * s.rows_lo, D)
    h0_hi = np.ascontiguousarray(h3d[:, s.rows_lo:]).reshape(M * s.rows_hi, D)
    in_maps = []
    for c in range(M):
        in_maps.append({
            "h0_lo": h0_lo, "h0_hi": h0_hi,
            "h0_slice": np.ascontiguousarray(h0p[c * rpc:(c + 1) * rpc]),
            "idx_lo": idx_sbuf_layout(s.idx_lo[c]),
            "idx_hi": idx_sbuf_layout(s.idx_hi[c]),
            "dvec": s.dvec[c].astype(DT_NP),
            "iota": iota,
            "ident": ident, "identdt": identdt,
            "w1t": w1t, "w2t": w2t, "gb": gb,
        })
    trace = bool(int(os.environ.get("KERNEL_TRACE", "0")))
    res = run_bass_kernel_spmd(nc, in_maps, core_ids=list(range(M)), trace=trace)
    LAST_EXEC_NS = res.exec_time_ns
    LAST_PROFILE = res.profile_json
    h = np.concatenate([res.results[c]["h3"][:npc] for c in range(M)], 0)
    return h


def kernel(feat, enc_mask_token, src, dst, ring_nodes, sub_src, sub_dst,
           on_W1, on_W2, on_g1, on_b1, on_g2, on_b2,
           tg_W1, tg_W2, tg_g1, tg_b1, tg_g2, tg_b2):
    feat = np.asarray(feat, np.float32)
    ring = np.asarray(ring_nodes, np.int64)
    rem = feat.copy()
    rem[ring] = np.asarray(enc_mask_token, np.float32)[0]

    n = feat.shape[0]
    s = build_structure(np.asarray(src), np.asarray(dst), n, n // M, 25)
    h1 = run_encoder_device(s, rem, dict(W1=np.asarray(on_W1), W2=np.asarray(on_W2),
                                         g1=np.asarray(on_g1), b1=np.asarray(on_b1),
                                         g2=np.asarray(on_g2), b2=np.asarray(on_b2)))

    h2 = _np_encoder(feat[ring], np.asarray(sub_src, np.int64),
                     np.asarray(sub_dst, np.int64),
                     np.asarray(tg_W1), np.asarray(tg_W2), np.asarray(tg_g1),
                     np.asarray(tg_b1), np.asarray(tg_g2), np.asarray(tg_b2))

    x = h1[ring]
    xn = x / np.maximum(np.linalg.norm(x, axis=-1, keepdims=True), 1e-12)
    yn = h2 / np.maximum(np.linalg.norm(h2, axis=-1, keepdims=True), 1e-12)
    return np.float32((1.0 - (xn * yn).sum(-1)).mean())

